# revision 34
# baseline (speedup 1.0000x reference)
"""ArcFace (AngularPenaltySMLoss) fused loss kernel for 8 Trainium2 NeuronCores.

Strategy: data-parallel over rows N (each core owns N/8 = 1024 rows of x,
streams the full W).  v2: three-engine drain.

  1. fp8(e4m3) DoubleRow matmul: host uploads W^T [128, 2, 10000] fp8 (scaled
     by SB) and x_n^T [128, 2, 1024] fp8 (row-normalized, scaled by SA;
     SA*SB = S = 30).  Each matmul contracts the full K=256 (two 128-planes)
     in one instruction at 2 cols/cycle (DoubleRow fp8 = 0.5 cyc/row).
     PSUM is an 8-half-slot ring pm[128, 8*512] f32 (all 16KB); matmul h of a
     row-block writes half-slot h%8 (512 cols; the last is 272: C=10000 is
     NOT padded, so no pad correction is needed anywhere).
  2. Drain = the bottleneck (exp over [128 rows x 10000 classes] per block).
     Three engines in parallel, plan per row-block j (h-chunk runs of 512):
       A (Scalar/ACT):  exp with fused row-sum accum_out, bf16 dump out.
       P (Pool/GpSimd): Schraudolph bf16 exp: int16(A16*x + B16) is the bit
                        pattern of ~exp(x) in bf16; writes idump (i16).
       D (Vector/DVE):  same Schraudolph TS for its share of chunks.
     All P/D chunks write a PACKED idump region; one DVE tensor_scalar over
     the bf16-bitcast of idump with accum_out (out=junk bf16) sums them in a
     single pass at the DVE 4x_2p rate (all-SBUF, all-16-bit).
  3. Target path: scaled target scores t_s = S*<x_n, W[target]> are host
     inputs (exact f32); device computes numerator = cosM*t_s -
     sinM*sqrt(S^2-t_s^2) with a Quake-rsqrt DVE chain, exp via f32
     Schraudolph, ln(denom) via a DVE bits->log2 approximation.
  4. W streams over two parallel HWDGE queues in sweep order; x^T lands
     first.  Per-core partial sum of L_i; host combines 8 scalars.
"""

import math

import numpy as np

S = 30.0
MARGIN = 0.3
EPS = 1e-7
N, D, C = 8192, 256, 10000
NCORES = 8
NLOC = N // NCORES  # 1024 rows per core
NJ = NLOC // 128  # 8 row-chunks of 128 partitions
HW = 512  # half-chunk width (one PSUM bank)
NH = (C + HW - 1) // HW  # 20 half-chunks per row-block
LASTW = C - (NH - 1) * HW  # 272 cols in the last half-chunk
SA = 8.0  # fp8 scale folded into normalized x
SB = 3.75  # fp8 scale folded into W  (SA*SB = S)

# f32 Schraudolph constants (target-score path only)
AEXP = 12102203.0
BEXP = 1064881816.0
# bf16 Schraudolph constants (bulk exp path): int16(A16*x + B16) bitcast to
# bf16 ~= exp(x); B16 tuned for zero exp-weighted mean error on x~N(0,1.5^2)
A16 = 184.66496580927726  # 128/ln2
B16 = 16248.642
RSQRT_MAGIC = 1597463007.0  # 0x5f3759df

# Per-row-block drain plan: (h_start, h_end, engine, acc_col_or_packed_off)
#   A -> ACT exp+accum into acc[:, j, col]
#   D -> Schraudolph TS into idump packed at given i16-column offset
# (GPSIMD/Pool cannot read PSUM on TRN2, so the drain is ACT+DVE only.)
# Drain plans (selectable via KPLAN env for experiments; default = best).
# ACT instrs fuse row-sum via accum_out; D chunks go through the bf16
# Schraudolph + 2x tree path.
import os as _os

_PLANS = {
    # v3f: ACT front 3 windows, DVE tail 4 narrow drains
    "front": [
        (0, 4, "A", 0),
        (4, 8, "A", 1),
        (8, 12, "A", 2),
        (12, 14, "D", 0),
        (14, 16, "D", 1024),
        (16, 18, "D", 2048),
        (18, 20, "D", 3072),
    ],
    # front + the 272 tail moved to ACT (rebalances DVE->ACT by ~5us)
    "front2": [
        (0, 4, "A", 0),
        (4, 8, "A", 1),
        (8, 12, "A", 2),
        (12, 14, "D", 0),
        (14, 16, "D", 1024),
        (16, 18, "D", 2048),
        (18, 19, "D", 3072),
        (19, 20, "A", 3),
    ],
    # v3g: slot-disjoint groups (ACT on X windows, DVE on Y windows)
    "xy": [
        (0, 4, "A", 0),
        (4, 6, "D", 0),
        (6, 8, "D", 1024),
        (8, 12, "A", 1),
        (12, 14, "D", 2048),
        (14, 16, "D", 3072),
        (16, 20, "A", 2),
    ],
    # interleaved, ACT split into 2048s at window starts
    "alt": [
        (0, 4, "A", 0),
        (4, 6, "D", 0),
        (6, 8, "D", 1024),
        (8, 12, "A", 1),
        (12, 16, "A", 2),
        (16, 18, "D", 2048),
        (18, 20, "D", 3072),
    ],
}
DRAIN_RUNS = _PLANS[_os.environ.get("KPLAN", "front")]
IDW = 4096  # packed idump region (tree input; zero-padded past DPACK)


def _run_width(h0, h1):
    return sum(LASTW if h == NH - 1 else HW for h in range(h0, h1))


DPACK = sum(_run_width(h0, h1) for h0, h1, eng, _ in DRAIN_RUNS if eng == "D")
NACC = 4  # acc cols for up to four A-runs; D-sums land in dleaf
NLEAF = 128  # bf16 tree reduces idump 4096 -> 128 leaves per row-block

_CACHE = {}


def _build():
    import concourse.bass as bass  # noqa: F401
    import concourse.mybir as mybir
    import concourse.tile as tile
    from concourse import bacc

    f32 = mybir.dt.float32
    bf16 = mybir.dt.bfloat16
    f8 = mybir.dt.float8e4
    i16 = mybir.dt.int16
    i32 = mybir.dt.int32
    AF = mybir.ActivationFunctionType
    OP = mybir.AluOpType
    DR = mybir.MatmulPerfMode.DoubleRow

    nc = bacc.Bacc()
    xT_ext = nc.declare_dram_parameter("xT", [128, 2, NLOC], f8, isOutput=False)
    wt_ext = nc.declare_dram_parameter("wt", [128, 2, C], f8, isOutput=False)
    ts_ext = nc.declare_dram_parameter("ts", [128, NJ], f32, isOutput=False)
    out_ext = nc.declare_dram_parameter("out", [1, 1], f32, isOutput=True)

    with tile.TileContext(nc) as tc:
        with (
            tc.tile_pool(name="singles", bufs=1) as singles,
            tc.tile_pool(name="idpool", bufs=3) as idpool,
            tc.tile_pool(name="pmain", bufs=1, space="PSUM") as psum_main,
        ):
            # whole PSUM as a flat 8-half-slot ring (8 banks x 512 f32)
            pm = psum_main.tile([128, 8 * HW], f32)

            # ---------------- loads (j=0 critical path first) ------------
            xT = singles.tile([128, 2, NLOC], f8)
            wt = singles.tile([128, 2, C], f8)
            traw = singles.tile([128, NJ], f32)  # t_s = S*t, host-computed
            nc.scalar.dma_start(out=xT, in_=xT_ext[:, :, :])
            W_ROUNDS = [(0, 2048), (2048, 4096), (4096, 6144), (6144, 8192), (8192, C)]
            for r, (c0, c1) in enumerate(W_ROUNDS):
                eng = nc.sync if r % 2 == 0 else nc.scalar
                eng.dma_start(out=wt[:, :, c0:c1], in_=wt_ext[:, :, c0:c1])
            nc.scalar.dma_start(out=traw, in_=ts_ext[:, :])

            rs_seed = singles.tile([128, NJ], i32)
            rs_t1 = singles.tile([128, NJ], f32)
            rs_y1 = singles.tile([128, NJ], f32)
            rs_t2 = singles.tile([128, NJ], f32)

            def rsqrt2(src, dst, fold=1.0):
                # Quake rsqrt + 2 Newton iterations; dst = fold/sqrt(src)
                nc.vector.tensor_scalar(
                    out=rs_seed,
                    in0=src.bitcast(i32),
                    scalar1=-0.5,
                    scalar2=RSQRT_MAGIC,
                    op0=OP.mult,
                    op1=OP.add,
                )
                y0 = rs_seed.bitcast(f32)
                nc.vector.tensor_tensor(out=rs_t1, in0=y0, in1=y0, op=OP.mult)
                nc.vector.tensor_tensor(out=rs_t1, in0=rs_t1, in1=src, op=OP.mult)
                nc.vector.tensor_scalar(
                    out=rs_t1, in0=rs_t1, scalar1=-0.5, scalar2=1.5,
                    op0=OP.mult, op1=OP.add,
                )
                nc.vector.tensor_tensor(out=rs_y1, in0=y0, in1=rs_t1, op=OP.mult)
                nc.vector.tensor_tensor(out=rs_t2, in0=rs_y1, in1=rs_y1, op=OP.mult)
                nc.vector.tensor_tensor(out=rs_t2, in0=rs_t2, in1=src, op=OP.mult)
                nc.vector.tensor_scalar(
                    out=rs_t2, in0=rs_t2, scalar1=-0.5 * fold, scalar2=1.5 * fold,
                    op0=OP.mult, op1=OP.add,
                )
                nc.vector.tensor_tensor(out=dst, in0=rs_y1, in1=rs_t2, op=OP.mult)

            def numer_chain():
                sclip = S * (1.0 - EPS)
                nc.vector.tensor_scalar(
                    out=tcl, in0=traw, scalar1=-sclip, scalar2=sclip,
                    op0=OP.max, op1=OP.min,
                )
                nc.vector.tensor_tensor(out=usq, in0=tcl, in1=tcl, op=OP.mult)
                nc.vector.tensor_scalar(
                    out=usq, in0=usq, scalar1=-1.0, scalar2=S * S,
                    op0=OP.mult, op1=OP.add,
                )
                # rtm = -sinM*sqrt(usq) = usq * (-sinM * rsqrt(usq))
                rsqrt2(usq, rsu, fold=-math.sin(MARGIN))
                nc.vector.tensor_tensor(out=rtm, in0=usq, in1=rsu, op=OP.mult)
                nc.vector.scalar_tensor_tensor(
                    out=numer, in0=tcl, scalar=math.cos(MARGIN), in1=rtm,
                    op0=OP.mult, op1=OP.add,
                )

            tcl = singles.tile([128, NJ], f32)
            usq = singles.tile([128, NJ], f32)
            rsu = singles.tile([128, NJ], f32)
            rtm = singles.tile([128, NJ], f32)
            numer = singles.tile([128, NJ], f32)
            exp_num = singles.tile([128, NJ], f32)
            exp_st = singles.tile([128, NJ], f32)

            # ---------------- main loop ----------------------------------
            acc = singles.tile([128, NJ, NACC], f32)
            nc.gpsimd.memset(acc, 0.0)
            edump = singles.tile([128, 2048], bf16)
            dleaf = singles.tile([128, NJ, NLEAF], bf16)
            tr = singles.tile([128, IDW], bf16)  # non-aliased tree scratch

            def tree_sum(idump_bf, j):
                # bf16 pairwise tree: level 1 on DVE at 2x_1p, the smaller
                # levels on the otherwise-idle Pool engine.  Each level
                # writes a fresh tr region (in-place aliasing and bitcast
                # inputs demote the DVE to 1x).
                w = IDW // 2
                src = idump_bf
                off = 0
                while w > NLEAF:
                    eng = nc.vector if w == IDW // 2 else nc.gpsimd
                    eng.tensor_tensor(
                        out=tr[:, off : off + w],
                        in0=src[:, :w],
                        in1=src[:, w : 2 * w],
                        op=OP.add,
                    )
                    src = tr[:, off : off + w]
                    off += w
                    w //= 2
                nc.gpsimd.tensor_tensor(
                    out=dleaf[:, j, :], in0=src[:, :w], in1=src[:, w : 2 * w],
                    op=OP.add,
                )

            def hcols(h, j):
                # (psum col offset, width) of half-chunk h; the slot map
                # rotates by 4 per row-block so consecutive blocks start on
                # opposite PSUM halves (kills the end-of-block drain stall)
                w = LASTW if h == NH - 1 else HW
                return ((h + 4 * j) % 8) * HW, w

            def run_ap(h0, h1, j):
                # contiguous psum AP covering half-chunks [h0, h1)
                off, _ = hcols(h0, j)
                w = sum(hcols(h, j)[1] for h in range(h0, h1))
                return pm[:, off : off + w], w

            for j in range(NJ):
                idump = idpool.tile([128, IDW], bf16, tag="id")
                if DPACK < IDW:
                    # keep the tree's zero pad intact (idle Pool engine)
                    nc.gpsimd.memset(idump[:, DPACK:IDW], 0.0)
                drain_at = {r[1] - 1: r for r in DRAIN_RUNS}
                for h in range(NH):
                    off, w = hcols(h, j)
                    nc.tensor.matmul(
                        out=pm[:, off : off + w],
                        lhsT=xT[:, :, j * 128 : (j + 1) * 128],
                        rhs=wt[:, :, h * HW : h * HW + w],
                        start=True,
                        stop=True,
                        perf_mode=DR,
                        skip_group_check=True,
                    )
                    if h in drain_at:
                        h0, h1, eng, arg = drain_at[h]
                        src, w_run = run_ap(h0, h1, j)
                        if eng == "A":
                            nc.scalar.activation(
                                out=edump[:, :w_run],
                                in_=src,
                                func=AF.Exp,
                                accum_out=acc[:, j, arg : arg + 1],
                            )
                        else:
                            nc.vector.tensor_scalar(
                                out=idump.bitcast(i16)[:, arg : arg + w_run],
                                in0=src,
                                scalar1=A16,
                                scalar2=B16,
                                op0=OP.mult,
                                op1=OP.add,
                            )
                # this block's tree goes at the END of the DVE queue: it runs
                # into the next block's DVE-idle front window, never delaying
                # a psum-freeing TS1
                tree_sum(idump, j)
                if j == 0:
                    # DVE target-path chain slots in behind the first sweep
                    numer_chain()
                elif j == 1:
                    nc.vector.tensor_scalar(
                        out=exp_num.bitcast(i32), in0=numer, scalar1=AEXP,
                        scalar2=BEXP, op0=OP.mult, op1=OP.add,
                    )
                    nc.vector.tensor_scalar(
                        out=exp_st.bitcast(i32), in0=tcl, scalar1=AEXP,
                        scalar2=BEXP, op0=OP.mult, op1=OP.add,
                    )

            # ---------------- combine ----------------
            rowsum = singles.tile([128, NJ], f32)
            dlsum = singles.tile([128, NJ], f32)
            dnum = singles.tile([128, NJ], f32)  # exp(numer) - exp(t_s)
            nc.vector.tensor_tensor(out=dnum, in0=exp_num, in1=exp_st, op=OP.subtract)
            nc.vector.tensor_reduce(
                out=rowsum, in_=acc, axis=mybir.AxisListType.X, op=OP.add
            )
            nc.vector.tensor_reduce(
                out=dlsum, in_=dleaf, axis=mybir.AxisListType.X, op=OP.add
            )
            nc.vector.tensor_tensor(out=rowsum, in0=rowsum, in1=dlsum, op=OP.add)
            denom = singles.tile([128, NJ], f32)
            nc.vector.tensor_tensor(out=denom, in0=rowsum, in1=dnum, op=OP.add)
            # ln(denom) on DVE: y = bits/2^23 - 127 = e + m;
            # ln(d) ~= ln2*(y + K2*m*(1-m)) with m = frac(y).
            K2 = 0.3398
            ly = singles.tile([128, NJ], f32)
            nc.vector.tensor_scalar(
                out=ly, in0=denom.bitcast(i32), scalar1=1.0 / (1 << 23),
                scalar2=-127.0, op0=OP.mult, op1=OP.add,
            )
            lyi = singles.tile([128, NJ], i32)
            nc.vector.tensor_scalar(
                out=lyi, in0=ly, scalar1=1.0, scalar2=None, op0=OP.mult
            )
            lm0 = singles.tile([128, NJ], f32)
            nc.vector.tensor_tensor(out=lm0, in0=ly, in1=lyi, op=OP.subtract)
            lneg = singles.tile([128, NJ], f32)
            nc.vector.tensor_scalar(
                out=lneg, in0=lm0, scalar1=0.0, scalar2=None, op0=OP.is_lt
            )
            lm = singles.tile([128, NJ], f32)
            nc.vector.tensor_tensor(out=lm, in0=lm0, in1=lneg, op=OP.add)
            lom = singles.tile([128, NJ], f32)
            nc.vector.tensor_scalar(
                out=lom, in0=lm, scalar1=-1.0, scalar2=1.0, op0=OP.mult, op1=OP.add
            )
            lq = singles.tile([128, NJ], f32)
            nc.vector.tensor_tensor(out=lq, in0=lm, in1=lom, op=OP.mult)
            la = singles.tile([128, NJ], f32)
            nc.vector.scalar_tensor_tensor(
                out=la, in0=lq, scalar=K2, in1=ly, op0=OP.mult, op1=OP.add
            )
            Lt = singles.tile([128, NJ], f32)
            nc.vector.scalar_tensor_tensor(
                out=Lt, in0=la, scalar=-math.log(2.0), in1=numer,
                op0=OP.mult, op1=OP.add,
            )
            Lrow = singles.tile([128, 1], f32)
            nc.vector.tensor_reduce(
                out=Lrow, in_=Lt, axis=mybir.AxisListType.X, op=OP.add
            )
            ones = singles.tile([128, 1], f32)
            nc.vector.memset(ones, 1.0)
            nc.tensor.matmul(
                out=pm[0:1, 0:1], lhsT=Lrow, rhs=ones, start=True, stop=True
            )
            Lp = singles.tile([1, 1], f32)
            nc.vector.tensor_copy(out=Lp, in_=pm[0:1, 0:1])
            nc.sync.dma_start(out=out_ext[:, :], in_=Lp)

    nc.finalize()
    return nc


def _get_nc():
    if "nc" not in _CACHE:
        _CACHE["nc"] = _build()
    return _CACHE["nc"]


def prepare_in_maps(x, W, target):
    import ml_dtypes

    f8 = ml_dtypes.float8_e4m3fn

    x = np.asarray(x, dtype=np.float32)
    W = np.asarray(W, dtype=np.float32)
    tgt = np.asarray(target).astype(np.int64).reshape(N)

    xn = x / np.linalg.norm(x, axis=1, keepdims=True)
    xna = (xn * np.float32(SA)).astype(np.float32)

    ws = W * np.float32(SB)
    # W^T in [partition(=d%128), plane(=d//128), class] fp8 layout
    wt = np.ascontiguousarray(
        ws.T.reshape(2, 128, C).transpose(1, 0, 2).astype(f8)
    )
    # scaled target scores t_s = S * <x_n, W[tgt]> (exact f32)
    ts_full = np.einsum("nd,nd->n", xna, ws[tgt]).astype(np.float32)

    in_maps = []
    for c in range(NCORES):
        sl = slice(c * NLOC, (c + 1) * NLOC)
        xs = xna[sl]
        in_maps.append(
            {
                # x_n^T fp8 [d%128, d//128, row]
                "xT": np.ascontiguousarray(
                    xs.T.reshape(2, 128, NLOC).transpose(1, 0, 2).astype(f8)
                ),
                "wt": wt,
                # t_s in [row%128, row//128] layout
                "ts": np.ascontiguousarray(ts_full[sl].reshape(NJ, 128).T),
            }
        )
    return in_maps


def kernel(x, W, target):
    from concourse.bass_utils import run_bass_kernel_spmd

    nc = _get_nc()
    in_maps = prepare_in_maps(x, W, target)
    res = run_bass_kernel_spmd(nc, in_maps, core_ids=list(range(NCORES)))
    parts = np.stack(
        [res.results[i]["out"].astype(np.float32).reshape(()) for i in range(NCORES)]
    )
    total = np.sum(parts, dtype=np.float32)
    return np.float32(-(total / np.float32(N)))


# revision 38
# speedup vs baseline: 1.0159x; 1.0159x over previous
"""ArcFace (AngularPenaltySMLoss) fused loss kernel for 8 Trainium2 NeuronCores.

Strategy: data-parallel over rows N (each core owns N/8 = 1024 rows of x,
streams the full W).  v2: three-engine drain.

  1. fp8(e4m3) DoubleRow matmul: host uploads W^T [128, 2, 10000] fp8 (scaled
     by SB) and x_n^T [128, 2, 1024] fp8 (row-normalized, scaled by SA;
     SA*SB = S = 30).  Each matmul contracts the full K=256 (two 128-planes)
     in one instruction at 2 cols/cycle (DoubleRow fp8 = 0.5 cyc/row).
     PSUM is an 8-half-slot ring pm[128, 8*512] f32 (all 16KB); matmul h of a
     row-block writes half-slot h%8 (512 cols; the last is 272: C=10000 is
     NOT padded, so no pad correction is needed anywhere).
  2. Drain = the bottleneck (exp over [128 rows x 10000 classes] per block).
     Three engines in parallel, plan per row-block j (h-chunk runs of 512):
       A (Scalar/ACT):  exp with fused row-sum accum_out, bf16 dump out.
       P (Pool/GpSimd): Schraudolph bf16 exp: int16(A16*x + B16) is the bit
                        pattern of ~exp(x) in bf16; writes idump (i16).
       D (Vector/DVE):  same Schraudolph TS for its share of chunks.
     All P/D chunks write a PACKED idump region; one DVE tensor_scalar over
     the bf16-bitcast of idump with accum_out (out=junk bf16) sums them in a
     single pass at the DVE 4x_2p rate (all-SBUF, all-16-bit).
  3. Target path: scaled target scores t_s = S*<x_n, W[target]> are host
     inputs (exact f32); device computes numerator = cosM*t_s -
     sinM*sqrt(S^2-t_s^2) with a Quake-rsqrt DVE chain, exp via f32
     Schraudolph, ln(denom) via a DVE bits->log2 approximation.
  4. W streams over two parallel HWDGE queues in sweep order; x^T lands
     first.  Per-core partial sum of L_i; host combines 8 scalars.
"""

import math

import numpy as np

S = 30.0
MARGIN = 0.3
EPS = 1e-7
N, D, C = 8192, 256, 10000
NCORES = 8
NLOC = N // NCORES  # 1024 rows per core
NJ = NLOC // 128  # 8 row-chunks of 128 partitions
HW = 512  # half-chunk width (one PSUM bank)
NH = (C + HW - 1) // HW  # 20 half-chunks per row-block
LASTW = C - (NH - 1) * HW  # 272 cols in the last half-chunk
SA = 8.0  # fp8 scale folded into normalized x
SB = 3.75  # fp8 scale folded into W  (SA*SB = S)

# f32 Schraudolph constants (target-score path only)
AEXP = 12102203.0
BEXP = 1064881816.0
# bf16 Schraudolph constants (bulk exp path): int16(A16*x + B16) bitcast to
# bf16 ~= exp(x); B16 tuned for zero exp-weighted mean error on x~N(0,1.5^2)
A16 = 184.66496580927726  # 128/ln2
B16 = 16248.642
RSQRT_MAGIC = 1597463007.0  # 0x5f3759df

# Per-row-block drain plan: (h_start, h_end, engine, acc_col_or_packed_off)
#   A -> ACT exp+accum into acc[:, j, col]
#   D -> Schraudolph TS into idump packed at given i16-column offset
# (GPSIMD/Pool cannot read PSUM on TRN2, so the drain is ACT+DVE only.)
# Drain plans (selectable via KPLAN env for experiments; default = best).
# ACT instrs fuse row-sum via accum_out; D chunks go through the bf16
# Schraudolph + 2x tree path.
import os as _os

_PLANS = {
    # v3f: ACT front 3 windows, DVE tail 4 narrow drains
    "front": [
        (0, 4, "A", 0),
        (4, 8, "A", 1),
        (8, 12, "A", 2),
        (12, 14, "D", 0),
        (14, 16, "D", 1024),
        (16, 18, "D", 2048),
        (18, 20, "D", 3072),
    ],
    # front + the 272 tail moved to ACT (rebalances DVE->ACT by ~5us)
    "front2": [
        (0, 4, "A", 0),
        (4, 8, "A", 1),
        (8, 12, "A", 2),
        (12, 14, "D", 0),
        (14, 16, "D", 1024),
        (16, 18, "D", 2048),
        (18, 19, "D", 3072),
        (19, 20, "A", 3),
    ],
    # v3g: slot-disjoint groups (ACT on X windows, DVE on Y windows)
    "xy": [
        (0, 4, "A", 0),
        (4, 6, "D", 0),
        (6, 8, "D", 1024),
        (8, 12, "A", 1),
        (12, 14, "D", 2048),
        (14, 16, "D", 3072),
        (16, 20, "A", 2),
    ],
    # interleaved, ACT split into 2048s at window starts
    "alt": [
        (0, 4, "A", 0),
        (4, 6, "D", 0),
        (6, 8, "D", 1024),
        (8, 12, "A", 1),
        (12, 16, "A", 2),
        (16, 18, "D", 2048),
        (18, 20, "D", 3072),
    ],
}
DRAIN_RUNS = _PLANS[_os.environ.get("KPLAN", "front")]
IDW = 4096  # packed idump region (tree input; zero-padded past DPACK)


def _run_width(h0, h1):
    return sum(LASTW if h == NH - 1 else HW for h in range(h0, h1))


DPACK = sum(_run_width(h0, h1) for h0, h1, eng, _ in DRAIN_RUNS if eng == "D")
NACC = 4  # acc cols for up to four A-runs; D-sums land in dleaf
NLEAF = 128  # bf16 tree reduces idump 4096 -> 128 leaves per row-block

_CACHE = {}


def _build():
    import concourse.bass as bass  # noqa: F401
    import concourse.mybir as mybir
    import concourse.tile as tile
    from concourse import bacc

    f32 = mybir.dt.float32
    bf16 = mybir.dt.bfloat16
    f8 = mybir.dt.float8e4
    i16 = mybir.dt.int16
    i32 = mybir.dt.int32
    AF = mybir.ActivationFunctionType
    OP = mybir.AluOpType
    DR = mybir.MatmulPerfMode.DoubleRow

    nc = bacc.Bacc()
    xT_ext = nc.declare_dram_parameter("xT", [128, 2, NLOC], f8, isOutput=False)
    wt_ext = nc.declare_dram_parameter("wt", [128, 2, C], f8, isOutput=False)
    ts_ext = nc.declare_dram_parameter("ts", [128, NJ], f32, isOutput=False)
    out_ext = nc.declare_dram_parameter("out", [1, 1], f32, isOutput=True)

    with tile.TileContext(nc) as tc:
        with (
            tc.tile_pool(name="singles", bufs=1) as singles,
            tc.tile_pool(name="idpool", bufs=3) as idpool,
            tc.tile_pool(name="pmain", bufs=1, space="PSUM") as psum_main,
        ):
            # whole PSUM as a flat 8-half-slot ring (8 banks x 512 f32)
            pm = psum_main.tile([128, 8 * HW], f32)

            # ---------------- loads (j=0 critical path first) ------------
            xT = singles.tile([128, 2, NLOC], f8)
            wt = singles.tile([128, 2, C], f8)
            traw = singles.tile([128, NJ], f32)  # t_s = S*t, host-computed
            nc.scalar.dma_start(out=xT, in_=xT_ext[:, :, :])
            W_ROUNDS = [
                (0, 1024), (1024, 2048), (2048, 4096), (4096, 6144),
                (6144, 8192), (8192, C),
            ]
            for r, (c0, c1) in enumerate(W_ROUNDS):
                eng = nc.sync if r % 2 == 0 else nc.scalar
                eng.dma_start(out=wt[:, :, c0:c1], in_=wt_ext[:, :, c0:c1])
            nc.scalar.dma_start(out=traw, in_=ts_ext[:, :])

            rs_seed = singles.tile([128, NJ], i32)
            rs_t1 = singles.tile([128, NJ], f32)
            rs_y1 = singles.tile([128, NJ], f32)
            rs_t2 = singles.tile([128, NJ], f32)

            def rsqrt2(src, dst, fold=1.0):
                # Quake rsqrt + 2 Newton iterations; dst = fold/sqrt(src)
                nc.vector.tensor_scalar(
                    out=rs_seed,
                    in0=src.bitcast(i32),
                    scalar1=-0.5,
                    scalar2=RSQRT_MAGIC,
                    op0=OP.mult,
                    op1=OP.add,
                )
                y0 = rs_seed.bitcast(f32)
                nc.vector.tensor_tensor(out=rs_t1, in0=y0, in1=y0, op=OP.mult)
                nc.vector.tensor_tensor(out=rs_t1, in0=rs_t1, in1=src, op=OP.mult)
                nc.vector.tensor_scalar(
                    out=rs_t1, in0=rs_t1, scalar1=-0.5, scalar2=1.5,
                    op0=OP.mult, op1=OP.add,
                )
                nc.vector.tensor_tensor(out=rs_y1, in0=y0, in1=rs_t1, op=OP.mult)
                nc.vector.tensor_tensor(out=rs_t2, in0=rs_y1, in1=rs_y1, op=OP.mult)
                nc.vector.tensor_tensor(out=rs_t2, in0=rs_t2, in1=src, op=OP.mult)
                nc.vector.tensor_scalar(
                    out=rs_t2, in0=rs_t2, scalar1=-0.5 * fold, scalar2=1.5 * fold,
                    op0=OP.mult, op1=OP.add,
                )
                nc.vector.tensor_tensor(out=dst, in0=rs_y1, in1=rs_t2, op=OP.mult)

            def numer_chain():
                sclip = S * (1.0 - EPS)
                nc.vector.tensor_scalar(
                    out=tcl, in0=traw, scalar1=-sclip, scalar2=sclip,
                    op0=OP.max, op1=OP.min,
                )
                nc.vector.tensor_tensor(out=usq, in0=tcl, in1=tcl, op=OP.mult)
                nc.vector.tensor_scalar(
                    out=usq, in0=usq, scalar1=-1.0, scalar2=S * S,
                    op0=OP.mult, op1=OP.add,
                )
                # rtm = -sinM*sqrt(usq) = usq * (-sinM * rsqrt(usq))
                rsqrt2(usq, rsu, fold=-math.sin(MARGIN))
                nc.vector.tensor_tensor(out=rtm, in0=usq, in1=rsu, op=OP.mult)
                nc.vector.scalar_tensor_tensor(
                    out=numer, in0=tcl, scalar=math.cos(MARGIN), in1=rtm,
                    op0=OP.mult, op1=OP.add,
                )

            tcl = singles.tile([128, NJ], f32)
            usq = singles.tile([128, NJ], f32)
            rsu = singles.tile([128, NJ], f32)
            rtm = singles.tile([128, NJ], f32)
            numer = singles.tile([128, NJ], f32)
            exp_num = singles.tile([128, NJ], f32)
            exp_st = singles.tile([128, NJ], f32)

            # ---------------- main loop ----------------------------------
            acc = singles.tile([128, NJ, NACC], f32)
            nc.gpsimd.memset(acc, 0.0)
            edump = singles.tile([128, 2048], bf16)
            dleaf = singles.tile([128, NJ, NLEAF], bf16)
            tr = singles.tile([128, IDW], bf16)  # non-aliased tree scratch

            def tree_sum(idump_bf, j):
                # bf16 pairwise tree: level 1 on DVE at 2x_1p, the smaller
                # levels on the otherwise-idle Pool engine.  Each level
                # writes a fresh tr region (in-place aliasing and bitcast
                # inputs demote the DVE to 1x).
                w = IDW // 2
                src = idump_bf
                off = 0
                while w > NLEAF:
                    nc.vector.tensor_tensor(
                        out=tr[:, off : off + w],
                        in0=src[:, :w],
                        in1=src[:, w : 2 * w],
                        op=OP.add,
                    )
                    src = tr[:, off : off + w]
                    off += w
                    w //= 2
                nc.vector.tensor_tensor(
                    out=dleaf[:, j, :], in0=src[:, :w], in1=src[:, w : 2 * w],
                    op=OP.add,
                )

            def hcols(h, j):
                # (psum col offset, width) of half-chunk h; the slot map
                # rotates by 4 per row-block so consecutive blocks start on
                # opposite PSUM halves (kills the end-of-block drain stall)
                w = LASTW if h == NH - 1 else HW
                return ((h + 4 * j) % 8) * HW, w

            def run_ap(h0, h1, j):
                # contiguous psum AP covering half-chunks [h0, h1)
                off, _ = hcols(h0, j)
                w = sum(hcols(h, j)[1] for h in range(h0, h1))
                return pm[:, off : off + w], w

            # p-state warmup: dummy DR matmuls on xT while W streams in, so
            # the Tensor engine enters row-block 0 at speed (results are
            # overwritten by the first real fills; nothing reads them)
            for _ in range(12):
                nc.tensor.matmul(
                    out=pm[:, 7 * HW : 8 * HW],
                    lhsT=xT[:, :, 0:128],
                    rhs=xT[:, :, 0:512],
                    start=True,
                    stop=True,
                    perf_mode=DR,
                    skip_group_check=True,
                )

            for j in range(NJ):
                idump = idpool.tile([128, IDW], bf16, tag="id")
                if DPACK < IDW:
                    # keep the tree's zero pad intact (idle Pool engine)
                    nc.gpsimd.memset(idump[:, DPACK:IDW], 0.0)
                drain_at = {r[1] - 1: r for r in DRAIN_RUNS}
                for h in range(NH):
                    off, w = hcols(h, j)
                    nc.tensor.matmul(
                        out=pm[:, off : off + w],
                        lhsT=xT[:, :, j * 128 : (j + 1) * 128],
                        rhs=wt[:, :, h * HW : h * HW + w],
                        start=True,
                        stop=True,
                        perf_mode=DR,
                        skip_group_check=True,
                    )
                    if h in drain_at:
                        h0, h1, eng, arg = drain_at[h]
                        src, w_run = run_ap(h0, h1, j)
                        if eng == "A":
                            nc.scalar.activation(
                                out=edump[:, :w_run],
                                in_=src,
                                func=AF.Exp,
                                accum_out=acc[:, j, arg : arg + 1],
                            )
                        else:
                            nc.vector.tensor_scalar(
                                out=idump.bitcast(i16)[:, arg : arg + w_run],
                                in0=src,
                                scalar1=A16,
                                scalar2=B16,
                                op0=OP.mult,
                                op1=OP.add,
                            )
                # this block's tree goes at the END of the DVE queue: it runs
                # into the next block's DVE-idle front window, never delaying
                # a psum-freeing TS1
                tree_sum(idump, j)
                if j == 0:
                    # DVE target-path chain slots in behind the first sweep
                    numer_chain()
                elif j == 1:
                    nc.vector.tensor_scalar(
                        out=exp_num.bitcast(i32), in0=numer, scalar1=AEXP,
                        scalar2=BEXP, op0=OP.mult, op1=OP.add,
                    )
                    nc.vector.tensor_scalar(
                        out=exp_st.bitcast(i32), in0=tcl, scalar1=AEXP,
                        scalar2=BEXP, op0=OP.mult, op1=OP.add,
                    )

            # ---------------- combine ----------------
            rowsum = singles.tile([128, NJ], f32)
            dlsum = singles.tile([128, NJ], f32)
            dnum = singles.tile([128, NJ], f32)  # exp(numer) - exp(t_s)
            nc.vector.tensor_tensor(out=dnum, in0=exp_num, in1=exp_st, op=OP.subtract)
            nc.vector.tensor_reduce(
                out=rowsum, in_=acc, axis=mybir.AxisListType.X, op=OP.add
            )
            nc.vector.tensor_reduce(
                out=dlsum, in_=dleaf, axis=mybir.AxisListType.X, op=OP.add
            )
            nc.vector.tensor_tensor(out=rowsum, in0=rowsum, in1=dlsum, op=OP.add)
            denom = singles.tile([128, NJ], f32)
            nc.vector.tensor_tensor(out=denom, in0=rowsum, in1=dnum, op=OP.add)
            # ln(denom) on DVE: y = bits/2^23 - 127 = e + m;
            # ln(d) ~= ln2*(y + K2*m*(1-m)) with m = frac(y).
            K2 = 0.3398
            ly = singles.tile([128, NJ], f32)
            nc.vector.tensor_scalar(
                out=ly, in0=denom.bitcast(i32), scalar1=1.0 / (1 << 23),
                scalar2=-127.0, op0=OP.mult, op1=OP.add,
            )
            lyi = singles.tile([128, NJ], i32)
            nc.vector.tensor_scalar(
                out=lyi, in0=ly, scalar1=1.0, scalar2=None, op0=OP.mult
            )
            lm0 = singles.tile([128, NJ], f32)
            nc.vector.tensor_tensor(out=lm0, in0=ly, in1=lyi, op=OP.subtract)
            lneg = singles.tile([128, NJ], f32)
            nc.vector.tensor_scalar(
                out=lneg, in0=lm0, scalar1=0.0, scalar2=None, op0=OP.is_lt
            )
            lm = singles.tile([128, NJ], f32)
            nc.vector.tensor_tensor(out=lm, in0=lm0, in1=lneg, op=OP.add)
            lom = singles.tile([128, NJ], f32)
            nc.vector.tensor_scalar(
                out=lom, in0=lm, scalar1=-1.0, scalar2=1.0, op0=OP.mult, op1=OP.add
            )
            lq = singles.tile([128, NJ], f32)
            nc.vector.tensor_tensor(out=lq, in0=lm, in1=lom, op=OP.mult)
            la = singles.tile([128, NJ], f32)
            nc.vector.scalar_tensor_tensor(
                out=la, in0=lq, scalar=K2, in1=ly, op0=OP.mult, op1=OP.add
            )
            Lt = singles.tile([128, NJ], f32)
            nc.vector.scalar_tensor_tensor(
                out=Lt, in0=la, scalar=-math.log(2.0), in1=numer,
                op0=OP.mult, op1=OP.add,
            )
            Lrow = singles.tile([128, 1], f32)
            nc.vector.tensor_reduce(
                out=Lrow, in_=Lt, axis=mybir.AxisListType.X, op=OP.add
            )
            ones = singles.tile([128, 1], f32)
            nc.vector.memset(ones, 1.0)
            nc.tensor.matmul(
                out=pm[0:1, 0:1], lhsT=Lrow, rhs=ones, start=True, stop=True
            )
            Lp = singles.tile([1, 1], f32)
            nc.vector.tensor_copy(out=Lp, in_=pm[0:1, 0:1])
            nc.sync.dma_start(out=out_ext[:, :], in_=Lp)

    nc.finalize()
    return nc


def _get_nc():
    if "nc" not in _CACHE:
        _CACHE["nc"] = _build()
    return _CACHE["nc"]


def prepare_in_maps(x, W, target):
    import ml_dtypes

    f8 = ml_dtypes.float8_e4m3fn

    x = np.asarray(x, dtype=np.float32)
    W = np.asarray(W, dtype=np.float32)
    tgt = np.asarray(target).astype(np.int64).reshape(N)

    xn = x / np.linalg.norm(x, axis=1, keepdims=True)
    xna = (xn * np.float32(SA)).astype(np.float32)

    ws = W * np.float32(SB)
    # W^T in [partition(=d%128), plane(=d//128), class] fp8 layout
    wt = np.ascontiguousarray(
        ws.T.reshape(2, 128, C).transpose(1, 0, 2).astype(f8)
    )
    # scaled target scores t_s = S * <x_n, W[tgt]> (exact f32)
    ts_full = np.einsum("nd,nd->n", xna, ws[tgt]).astype(np.float32)

    in_maps = []
    for c in range(NCORES):
        sl = slice(c * NLOC, (c + 1) * NLOC)
        xs = xna[sl]
        in_maps.append(
            {
                # x_n^T fp8 [d%128, d//128, row]
                "xT": np.ascontiguousarray(
                    xs.T.reshape(2, 128, NLOC).transpose(1, 0, 2).astype(f8)
                ),
                "wt": wt,
                # t_s in [row%128, row//128] layout
                "ts": np.ascontiguousarray(ts_full[sl].reshape(NJ, 128).T),
            }
        )
    return in_maps


def kernel(x, W, target):
    from concourse.bass_utils import run_bass_kernel_spmd

    nc = _get_nc()
    in_maps = prepare_in_maps(x, W, target)
    res = run_bass_kernel_spmd(nc, in_maps, core_ids=list(range(NCORES)))
    parts = np.stack(
        [res.results[i]["out"].astype(np.float32).reshape(()) for i in range(NCORES)]
    )
    total = np.sum(parts, dtype=np.float32)
    return np.float32(-(total / np.float32(N)))
